# revision 2
# baseline (speedup 1.0000x reference)
"""Trainium2 Bass kernel for the DSA-MoE routing module.

Strategy: data-parallel over batch. Each of the 8 NeuronCores gets 2 full
batches (4096 tokens). Expert weights are replicated, cast to bf16 on host,
and kept SBUF-resident. Per core:

  - cond router: sum features over seq via DVE reduce on the pre-transposed
    bf16 activations, then a small fp32 matmul against cond_w/2048 (+cond_b
    via a rank-1 matmul). Final softmax happens on host from the returned
    logits; an on-device softmax produces the internal routing weights.
  - stage router: x_raw is tiny (16x2048x14) -> host computes stage softmax
    and passes the per-batch weights in.
  - main pipeline per 256-token block: GEMM1 (X.T @ W_down -> H.T in
    [EH, tokens] layout, bf16, PSUM fp32), fused Gelu+down_b eviction on
    ACT, per-(batch,expert) scale on DVE, GEMM2 accumulates
    delta = Hw.T^T @ W_up directly in PSUM across all 9 EH-chunks
    (plus a rank-1 matmul adding the up_b bias term), final eviction adds
    the fp32 residual.

The full (unsharded) inputs come in; sharding/gather happens on host.
"""

import sys

sys.path.insert(0, "/opt/trn_rl_repo")

from contextlib import ExitStack

import ml_dtypes
import numpy as np

import concourse.bass as bass  # noqa: F401  (registers bass types)
import concourse.tile as tile
from concourse import bacc, mybir
from concourse.bass_utils import run_bass_kernel_spmd

BF16, F32 = mybir.dt.bfloat16, mybir.dt.float32
AF = mybir.ActivationFunctionType
AX = mybir.AxisListType
ALU = mybir.AluOpType
BF16_NP = ml_dtypes.bfloat16

B, S, DM, HID = 16, 2048, 1024, 256
C, G = 6, 3
E = C * G                      # 18 experts
EH = E * HID                   # 4608
NCORE = 8
BPC = B // NCORE               # batches per core = 2
TPC = BPC * S                  # tokens per core = 4096
TB = 256                       # token block
NBLK = S // TB                 # blocks per batch = 8
KD = DM // 128                 # 8 k-tiles over D
NCH = 9                        # EH chunks
CHW = EH // NCH                # 512 EH cols per chunk
NKEH = EH // 128               # 36 EH k-tiles

_CACHE = {}
LAST_RESULT = None


def _build():
    nc = bacc.Bacc("TRN2", target_bir_lowering=False, debug=False,
                   num_devices=NCORE)
    xt = nc.dram_tensor("xt", [DM, TPC], BF16, kind="ExternalInput").ap()
    xres = nc.dram_tensor("xres", [TPC, DM], F32, kind="ExternalInput").ap()
    wdn = nc.dram_tensor("wdn", [DM, EH], BF16, kind="ExternalInput").ap()
    wup = nc.dram_tensor("wup", [EH, DM], BF16, kind="ExternalInput").ap()
    dbt = nc.dram_tensor("dbt", [128, NKEH], F32, kind="ExternalInput").ap()
    upb = nc.dram_tensor("upb", [E, DM], F32, kind="ExternalInput").ap()
    cws = nc.dram_tensor("cws", [DM, C], F32, kind="ExternalInput").ap()
    cb = nc.dram_tensor("cb", [1, C], F32, kind="ExternalInput").ap()
    sw = nc.dram_tensor("sw", [1, BPC * G], F32, kind="ExternalInput").ap()
    out = nc.dram_tensor("out", [TPC, DM], F32, kind="ExternalOutput").ap()
    clg = nc.dram_tensor("clg", [BPC, C], F32, kind="ExternalOutput").ap()

    with tile.TileContext(nc) as tc, ExitStack() as ctx:
        const = ctx.enter_context(tc.tile_pool(name="const", bufs=1))
        wpool = ctx.enter_context(tc.tile_pool(name="wpool", bufs=1))
        xtp = ctx.enter_context(tc.tile_pool(name="xtp", bufs=2))
        xrp = ctx.enter_context(tc.tile_pool(name="xrp", bufs=1))
        htp = ctx.enter_context(tc.tile_pool(name="htp", bufs=2))
        outp = ctx.enter_context(tc.tile_pool(name="outp", bufs=2))
        ps_g1 = ctx.enter_context(tc.tile_pool(name="psg1", bufs=3, space="PSUM"))
        ps_d = ctx.enter_context(tc.tile_pool(name="psd", bufs=1, space="PSUM"))
        ps_s = ctx.enter_context(tc.tile_pool(name="pss", bufs=1, space="PSUM"))

        # ---- resident weights -------------------------------------------
        wdn_sb = wpool.tile([128, KD * EH], BF16)       # k-tile k at [:, k*EH:]
        for k in range(KD):
            nc.sync.dma_start(out=wdn_sb[:, k * EH:(k + 1) * EH],
                              in_=wdn[k * 128:(k + 1) * 128, :])
        wup_sb = wpool.tile([128, NKEH * DM], BF16)     # k-tile kk at [:, kk*DM:]
        for kk in range(NKEH):
            nc.sync.dma_start(out=wup_sb[:, kk * DM:(kk + 1) * DM],
                              in_=wup[kk * 128:(kk + 1) * 128, :])

        dbt_sb = const.tile([128, NKEH], F32)
        nc.sync.dma_start(out=dbt_sb[:], in_=dbt[:])
        upb_sb = const.tile([E, DM], F32)
        nc.sync.dma_start(out=upb_sb[:], in_=upb[:])
        cws_sb = const.tile([128, KD * C], F32)
        for k in range(KD):
            nc.sync.dma_start(out=cws_sb[:, k * C:(k + 1) * C],
                              in_=cws[k * 128:(k + 1) * 128, :])
        cb_sb = const.tile([1, C], F32)
        nc.sync.dma_start(out=cb_sb[:], in_=cb[:])
        sw_sb = const.tile([1, BPC * G], F32)
        nc.sync.dma_start(out=sw_sb[:], in_=sw[:])
        ones_f = const.tile([1, 128], F32)
        nc.any.memset(ones_f[:], 1.0)
        ones_bf = const.tile([1, 128], BF16)
        nc.any.memset(ones_bf[:], 1.0)

        # ---- feature means (both batches) -------------------------------
        mf_sb = const.tile([128, BPC * KD], F32)
        for b in range(BPC):
            for k in range(KD):
                xtm = xtp.tile([128, S], BF16, tag="xtm")
                nc.sync.dma_start(out=xtm[:],
                                  in_=xt[k * 128:(k + 1) * 128,
                                         b * S:(b + 1) * S])
                nc.vector.reduce_sum(mf_sb[:, b * KD + k:b * KD + k + 1],
                                     xtm[:], axis=AX.X)

        # ---- routers per batch ------------------------------------------
        wsb = []
        bias_bf = []
        for b in range(BPC):
            lgp = ps_s.tile([1, C], F32, tag="small")
            for k in range(KD):
                nc.tensor.matmul(lgp[:], lhsT=mf_sb[:, b * KD + k:b * KD + k + 1],
                                 rhs=cws_sb[:, k * C:(k + 1) * C],
                                 start=(k == 0), stop=False)
            nc.tensor.matmul(lgp[:], lhsT=ones_f[0:1, 0:1], rhs=cb_sb[:],
                             start=False, stop=True)
            lg_sb = const.tile([1, C], F32, tag=f"lg{b}")
            nc.scalar.activation(lg_sb[:], lgp[:], AF.Copy)
            nc.sync.dma_start(out=clg[b:b + 1, :], in_=lg_sb[:])

            e_sb = const.tile([1, C], F32, tag=f"esb{b}")
            nc.scalar.activation(e_sb[:], lg_sb[:], AF.Exp)
            s_sb = const.tile([1, 1], F32, tag=f"ssb{b}")
            nc.vector.reduce_sum(s_sb[:], e_sb[:], axis=AX.X)
            r_sb = const.tile([1, 1], F32, tag=f"rsb{b}")
            nc.vector.reciprocal(r_sb[:], s_sb[:])
            cwd = const.tile([1, C], F32, tag=f"cwd{b}")
            nc.vector.tensor_scalar_mul(cwd[:], e_sb[:], r_sb[0:1, 0:1])

            # flat joint weights fw[e = c*G + g] = cond[c] * stage[g]
            fw = const.tile([1, E], F32, tag=f"fw{b}")
            fw3 = fw[0:1, :].rearrange("p (c g) -> p c g", g=G)
            cwd3 = cwd[0:1, :].rearrange("p (c u) -> p c u", u=1)
            for g in range(G):
                nc.vector.tensor_scalar_mul(fw3[:, :, g:g + 1], cwd3[:],
                                            sw_sb[0:1, b * G + g:b * G + g + 1])

            # broadcast across partitions: [128, E]
            wbp = ps_s.tile([128, E], F32, tag="small")
            nc.tensor.matmul(wbp[:], lhsT=ones_f[:], rhs=fw[:],
                             start=True, stop=True)
            ws = const.tile([128, E], F32, tag=f"ws{b}")
            nc.scalar.activation(ws[:], wbp[:], AF.Copy)
            wsb.append(ws)

            # fw -> column layout [E, 1] via DVE 32x32 transpose
            tin = const.tile([32, 32], F32, tag=f"ti{b}")
            nc.any.memset(tin[:], 0.0)
            nc.vector.tensor_copy(tin[0:1, 0:E], fw[:])
            tco = const.tile([32, 32], F32, tag=f"tc{b}")
            nc.vector.transpose(tco[:], tin[:])

            # bias_term = fw @ up_b   -> [1, DM], cast to bf16
            bb = const.tile([1, DM], BF16, tag=f"bb{b}")
            for hh in range(2):
                bps = ps_s.tile([1, 512], F32, tag="small")
                nc.tensor.matmul(bps[:], lhsT=tco[0:E, 0:1],
                                 rhs=upb_sb[0:E, hh * 512:(hh + 1) * 512],
                                 start=True, stop=True)
                nc.scalar.activation(bb[0:1, hh * 512:(hh + 1) * 512], bps[:],
                                     AF.Copy)
            bias_bf.append(bb)

        # ---- main pipeline ----------------------------------------------
        for b in range(BPC):
            for blk in range(NBLK):
                t0 = b * S + blk * TB
                xtb = xtp.tile([128, KD * TB], BF16, tag="xtm")
                for k in range(KD):
                    nc.sync.dma_start(out=xtb[:, k * TB:(k + 1) * TB],
                                      in_=xt[k * 128:(k + 1) * 128, t0:t0 + TB])
                xrb = xrp.tile([128, 2 * DM], F32)
                for tt in range(2):
                    nc.sync.dma_start(
                        out=xrb[:, tt * DM:(tt + 1) * DM],
                        in_=xres[t0 + tt * 128:t0 + (tt + 1) * 128, :])

                dps = [ps_d.tile([128, 512], F32, tag=f"d{i}", name=f"dps{i}")
                       for i in range(4)]
                for tt in range(2):
                    for hh in range(2):
                        nc.tensor.matmul(
                            dps[tt * 2 + hh][:], lhsT=ones_bf[:],
                            rhs=bias_bf[b][0:1, hh * 512:(hh + 1) * 512],
                            start=True, stop=False)

                for c in range(NCH):
                    htc = htp.tile([128, 4 * TB], BF16)
                    for m in range(4):
                        g1 = ps_g1.tile([128, TB], F32, tag="g1")
                        col0 = c * CHW + m * 128
                        for k in range(KD):
                            nc.tensor.matmul(
                                g1[:],
                                lhsT=wdn_sb[:, k * EH + col0:k * EH + col0 + 128],
                                rhs=xtb[:, k * TB:(k + 1) * TB],
                                start=(k == 0), stop=(k == KD - 1))
                        eh_t = c * 4 + m
                        hslice = htc[:, m * TB:(m + 1) * TB]
                        nc.scalar.activation(hslice, g1[:], AF.Gelu,
                                             bias=dbt_sb[:, eh_t:eh_t + 1])
                        e_idx = (c * CHW + m * 128) // HID
                        nc.vector.tensor_scalar_mul(
                            hslice, hslice, wsb[b][:, e_idx:e_idx + 1])
                    for tt in range(2):
                        for m in range(4):
                            lhs = htc[:, m * TB + tt * 128:m * TB + (tt + 1) * 128]
                            kk = c * 4 + m
                            for hh in range(2):
                                nc.tensor.matmul(
                                    dps[tt * 2 + hh][:], lhsT=lhs,
                                    rhs=wup_sb[:, kk * DM + hh * 512:
                                               kk * DM + (hh + 1) * 512],
                                    start=False,
                                    stop=(c == NCH - 1 and m == 3))

                outb = outp.tile([128, 2 * DM], F32)
                for tt in range(2):
                    for hh in range(2):
                        sl = slice(tt * DM + hh * 512, tt * DM + (hh + 1) * 512)
                        nc.vector.tensor_tensor(
                            out=outb[:, sl], in0=dps[tt * 2 + hh][:],
                            in1=xrb[:, sl], op=ALU.add)
                for tt in range(2):
                    nc.sync.dma_start(
                        out=out[t0 + tt * 128:t0 + (tt + 1) * 128, :],
                        in_=outb[:, tt * DM:(tt + 1) * DM])

    nc.compile()
    return nc


def _get_nc():
    if "nc" not in _CACHE:
        _CACHE["nc"] = _build()
    return _CACHE["nc"]


def _softmax32(x):
    x = np.asarray(x, np.float32)
    m = x.max(axis=-1, keepdims=True)
    e = np.exp(x - m)
    return e / e.sum(axis=-1, keepdims=True)


def kernel(**inputs):
    global LAST_RESULT
    features = np.ascontiguousarray(np.asarray(inputs["features"], np.float32))
    x_raw = np.asarray(inputs["x_raw"], np.float32)
    down_w = np.asarray(inputs["down_w"], np.float32)
    down_b = np.asarray(inputs["down_b"], np.float32)
    up_w = np.asarray(inputs["up_w"], np.float32)
    up_b = np.asarray(inputs["up_b"], np.float32)
    cond_w = np.asarray(inputs["cond_w"], np.float32)
    cond_b = np.asarray(inputs["cond_b"], np.float32)
    stage_w = np.asarray(inputs["stage_w"], np.float32)
    stage_b = np.asarray(inputs["stage_b"], np.float32)

    # host-side router for the tiny stage branch (16x2048x14 input)
    stage_logits = x_raw.mean(axis=1, dtype=np.float32) @ stage_w + stage_b
    stage_weights = _softmax32(stage_logits)            # [B, G]

    wdn_np = np.ascontiguousarray(
        down_w.transpose(1, 0, 2).reshape(DM, EH)).astype(BF16_NP)
    wup_np = np.ascontiguousarray(up_w.reshape(EH, DM)).astype(BF16_NP)
    dbt_np = np.ascontiguousarray(down_b.reshape(EH).reshape(NKEH, 128).T)
    upb_np = np.ascontiguousarray(up_b)                  # [E, DM]
    cws_np = np.ascontiguousarray(cond_w / np.float32(S))
    cb_np = cond_b.reshape(1, C)

    nc = _get_nc()
    in_maps = []
    for cidx in range(NCORE):
        fs = features[BPC * cidx:BPC * (cidx + 1)].reshape(TPC, DM)
        in_maps.append({
            "xt": fs.T.astype(BF16_NP),
            "xres": fs,
            "wdn": wdn_np,
            "wup": wup_np,
            "dbt": dbt_np,
            "upb": upb_np,
            "cws": cws_np,
            "cb": cb_np,
            "sw": np.ascontiguousarray(
                stage_weights[BPC * cidx:BPC * (cidx + 1)].reshape(1, BPC * G)),
        })

    res = run_bass_kernel_spmd(nc, in_maps, list(range(NCORE)))
    LAST_RESULT = res

    output = np.concatenate(
        [res.results[c]["out"] for c in range(NCORE)], axis=0
    ).reshape(B, S, DM)
    cond_logits = np.concatenate(
        [res.results[c]["clg"] for c in range(NCORE)], axis=0)   # [B, C]
    cond_weights = _softmax32(cond_logits)

    joint = cond_weights[:, :, None] * stage_weights[:, None, :]
    flat = joint.reshape(B, E).astype(np.float32)
    expert_loads = flat.mean(axis=0, dtype=np.float32)
    lb_loss = np.float32(E * np.sum(expert_loads * expert_loads,
                                    dtype=np.float32) * np.float32(0.01))

    return (output.astype(np.float32), cond_weights.astype(np.float32),
            stage_weights.astype(np.float32), expert_loads, lb_loss)


# revision 4
# speedup vs baseline: 1.0623x; 1.0623x over previous
"""Trainium2 Bass kernel for the DSA-MoE routing module.

Strategy: data-parallel over batch. Each of the 8 NeuronCores gets 2 full
batches (4096 tokens). Expert weights are replicated, cast to bf16 on host,
and kept SBUF-resident. Per core:

  - cond router: sum features over seq via DVE reduce on the pre-transposed
    bf16 activations, then a small fp32 matmul against cond_w/2048 (+cond_b
    via a rank-1 matmul). Final softmax happens on host from the returned
    logits; an on-device softmax produces the internal routing weights.
  - stage router: x_raw is tiny (16x2048x14) -> host computes stage softmax
    and passes the per-batch weights in.
  - main pipeline per 256-token block: GEMM1 (X.T @ W_down -> H.T in
    [EH, tokens] layout, bf16, PSUM fp32), fused Gelu+down_b eviction on
    ACT, per-(batch,expert) scale on DVE, GEMM2 accumulates
    delta = Hw.T^T @ W_up directly in PSUM across all 9 EH-chunks
    (plus a rank-1 matmul adding the up_b bias term), final eviction adds
    the fp32 residual.

The full (unsharded) inputs come in; sharding/gather happens on host.
"""

import sys

sys.path.insert(0, "/opt/trn_rl_repo")

from contextlib import ExitStack

import ml_dtypes
import numpy as np

import concourse.bass as bass  # noqa: F401  (registers bass types)
import concourse.tile as tile
from concourse import bacc, mybir
from concourse.bass_utils import run_bass_kernel_spmd

BF16, F32 = mybir.dt.bfloat16, mybir.dt.float32
AF = mybir.ActivationFunctionType
AX = mybir.AxisListType
ALU = mybir.AluOpType
BF16_NP = ml_dtypes.bfloat16

B, S, DM, HID = 16, 2048, 1024, 256
C, G = 6, 3
E = C * G                      # 18 experts
EH = E * HID                   # 4608
NCORE = 8
BPC = B // NCORE               # batches per core = 2
TPC = BPC * S                  # tokens per core = 4096
TB = 256                       # token block
NBLK = S // TB                 # blocks per batch = 8
KD = DM // 128                 # 8 k-tiles over D
NCH = 9                        # EH chunks
CHW = EH // NCH                # 512 EH cols per chunk
NKEH = EH // 128               # 36 EH k-tiles

_CACHE = {}
LAST_RESULT = None


def _build():
    nc = bacc.Bacc("TRN2", target_bir_lowering=False, debug=False,
                   num_devices=NCORE)
    xt = nc.dram_tensor("xt", [DM, TPC], BF16, kind="ExternalInput").ap()
    xres = nc.dram_tensor("xres", [TPC, DM], F32, kind="ExternalInput").ap()
    wdn = nc.dram_tensor("wdn", [DM, EH], BF16, kind="ExternalInput").ap()
    wup = nc.dram_tensor("wup", [EH, DM], BF16, kind="ExternalInput").ap()
    dbt = nc.dram_tensor("dbt", [128, NKEH], F32, kind="ExternalInput").ap()
    upb = nc.dram_tensor("upb", [E, DM], F32, kind="ExternalInput").ap()
    cws = nc.dram_tensor("cws", [DM, C], F32, kind="ExternalInput").ap()
    cb = nc.dram_tensor("cb", [1, C], F32, kind="ExternalInput").ap()
    sw = nc.dram_tensor("sw", [1, BPC * G], F32, kind="ExternalInput").ap()
    out = nc.dram_tensor("out", [TPC, DM], F32, kind="ExternalOutput").ap()
    clg = nc.dram_tensor("clg", [BPC, C], F32, kind="ExternalOutput").ap()

    with tile.TileContext(nc) as tc, ExitStack() as ctx:
        const = ctx.enter_context(tc.tile_pool(name="const", bufs=1))
        wpool = ctx.enter_context(tc.tile_pool(name="wpool", bufs=1))
        xtp = ctx.enter_context(tc.tile_pool(name="xtp", bufs=2))
        xrp = ctx.enter_context(tc.tile_pool(name="xrp", bufs=1))
        htp = ctx.enter_context(tc.tile_pool(name="htp", bufs=3))
        outp = ctx.enter_context(tc.tile_pool(name="outp", bufs=1))
        ps_g1 = ctx.enter_context(tc.tile_pool(name="psg1", bufs=3, space="PSUM"))
        ps_d = ctx.enter_context(tc.tile_pool(name="psd", bufs=1, space="PSUM"))
        ps_s = ctx.enter_context(tc.tile_pool(name="pss", bufs=1, space="PSUM"))

        # DMA issue order is roughly execution order: wdn + block-0
        # activations first (gives PE its runway), then the router means,
        # then wup (trickles in under block 0's GEMM1), then the rest.
        wdn_sb = wpool.tile([128, KD * EH], BF16)       # k-tile k at [:, k*EH:]
        for k in range(KD):
            nc.sync.dma_start(out=wdn_sb[:, k * EH:(k + 1) * EH],
                              in_=wdn[k * 128:(k + 1) * 128, :])
        xtb0 = xtp.tile([128, KD * TB], BF16, tag="xtm", name="xtb0")
        for k in range(KD):
            nc.sync.dma_start(out=xtb0[:, k * TB:(k + 1) * TB],
                              in_=xt[k * 128:(k + 1) * 128, 0:TB])
        dbt_sb = const.tile([128, NKEH], F32)
        nc.sync.dma_start(out=dbt_sb[:], in_=dbt[:])
        cws_sb = const.tile([128, KD * C], F32)
        for k in range(KD):
            nc.sync.dma_start(out=cws_sb[:, k * C:(k + 1) * C],
                              in_=cws[k * 128:(k + 1) * 128, :])
        cb_sb = const.tile([1, C], F32)
        nc.sync.dma_start(out=cb_sb[:], in_=cb[:])
        sw_sb = const.tile([1, BPC * G], F32)
        nc.sync.dma_start(out=sw_sb[:], in_=sw[:])
        ones_f = const.tile([1, 128], F32)
        nc.any.memset(ones_f[:], 1.0)
        ones_bf = const.tile([1, 128], BF16)
        nc.any.memset(ones_bf[:], 1.0)

        # ---- feature means (both batches) -------------------------------
        mf_sb = const.tile([128, BPC * KD], F32)
        for b in range(BPC):
            for k in range(KD):
                xtm = xtp.tile([128, S], BF16, tag="xmean")
                nc.sync.dma_start(out=xtm[:],
                                  in_=xt[k * 128:(k + 1) * 128,
                                         b * S:(b + 1) * S])
                nc.vector.reduce_sum(mf_sb[:, b * KD + k:b * KD + k + 1],
                                     xtm[:], axis=AX.X)

        wup_sb = wpool.tile([128, NKEH * DM], BF16)     # k-tile kk at [:, kk*DM:]
        for kk in range(NKEH):
            nc.sync.dma_start(out=wup_sb[:, kk * DM:(kk + 1) * DM],
                              in_=wup[kk * 128:(kk + 1) * 128, :])
        xrb0 = xrp.tile([128, 2 * DM], F32, name="xrb0")
        for tt in range(2):
            nc.sync.dma_start(out=xrb0[:, tt * DM:(tt + 1) * DM],
                              in_=xres[tt * 128:(tt + 1) * 128, :])
        upb_sb = const.tile([E, DM], F32)
        nc.sync.dma_start(out=upb_sb[:], in_=upb[:])

        # ---- routers per batch ------------------------------------------
        wsb = []
        bias_bf = []
        for b in range(BPC):
            lgp = ps_s.tile([1, C], F32, tag="small")
            for k in range(KD):
                nc.tensor.matmul(lgp[:], lhsT=mf_sb[:, b * KD + k:b * KD + k + 1],
                                 rhs=cws_sb[:, k * C:(k + 1) * C],
                                 start=(k == 0), stop=False)
            nc.tensor.matmul(lgp[:], lhsT=ones_f[0:1, 0:1], rhs=cb_sb[:],
                             start=False, stop=True)
            lg_sb = const.tile([1, C], F32, tag=f"lg{b}")
            nc.scalar.activation(lg_sb[:], lgp[:], AF.Copy)
            nc.sync.dma_start(out=clg[b:b + 1, :], in_=lg_sb[:])

            e_sb = const.tile([1, C], F32, tag=f"esb{b}")
            nc.scalar.activation(e_sb[:], lg_sb[:], AF.Exp)
            s_sb = const.tile([1, 1], F32, tag=f"ssb{b}")
            nc.vector.reduce_sum(s_sb[:], e_sb[:], axis=AX.X)
            r_sb = const.tile([1, 1], F32, tag=f"rsb{b}")
            nc.vector.reciprocal(r_sb[:], s_sb[:])
            cwd = const.tile([1, C], F32, tag=f"cwd{b}")
            nc.vector.tensor_scalar_mul(cwd[:], e_sb[:], r_sb[0:1, 0:1])

            # flat joint weights fw[e = c*G + g] = cond[c] * stage[g]
            fw = const.tile([1, E], F32, tag=f"fw{b}")
            fw3 = fw[0:1, :].rearrange("p (c g) -> p c g", g=G)
            cwd3 = cwd[0:1, :].rearrange("p (c u) -> p c u", u=1)
            for g in range(G):
                nc.vector.tensor_scalar_mul(fw3[:, :, g:g + 1], cwd3[:],
                                            sw_sb[0:1, b * G + g:b * G + g + 1])

            # broadcast across partitions: [128, E]
            wbp = ps_s.tile([128, E], F32, tag="small")
            nc.tensor.matmul(wbp[:], lhsT=ones_f[:], rhs=fw[:],
                             start=True, stop=True)
            ws = const.tile([128, E], F32, tag=f"ws{b}")
            nc.scalar.activation(ws[:], wbp[:], AF.Copy)
            wsb.append(ws)

            # fw -> column layout [E, 1] via DVE 32x32 transpose
            tin = const.tile([32, 32], F32, tag=f"ti{b}")
            nc.any.memset(tin[:], 0.0)
            nc.vector.tensor_copy(tin[0:1, 0:E], fw[:])
            tco = const.tile([32, 32], F32, tag=f"tc{b}")
            nc.vector.transpose(tco[:], tin[:])

            # bias_term = fw @ up_b   -> [1, DM], cast to bf16
            bb = const.tile([1, DM], BF16, tag=f"bb{b}")
            for hh in range(2):
                bps = ps_s.tile([1, 512], F32, tag="small")
                nc.tensor.matmul(bps[:], lhsT=tco[0:E, 0:1],
                                 rhs=upb_sb[0:E, hh * 512:(hh + 1) * 512],
                                 start=True, stop=True)
                nc.scalar.activation(bb[0:1, hh * 512:(hh + 1) * 512], bps[:],
                                     AF.Copy)
            bias_bf.append(bb)

        # ---- main pipeline ----------------------------------------------
        for b in range(BPC):
            for blk in range(NBLK):
                t0 = b * S + blk * TB
                if b == 0 and blk == 0:
                    xtb, xrb = xtb0, xrb0
                else:
                    xtb = xtp.tile([128, KD * TB], BF16, tag="xtm")
                    for k in range(KD):
                        nc.sync.dma_start(
                            out=xtb[:, k * TB:(k + 1) * TB],
                            in_=xt[k * 128:(k + 1) * 128, t0:t0 + TB])
                    xrb = xrp.tile([128, 2 * DM], F32)
                    for tt in range(2):
                        nc.sync.dma_start(
                            out=xrb[:, tt * DM:(tt + 1) * DM],
                            in_=xres[t0 + tt * 128:t0 + (tt + 1) * 128, :])

                dps = [ps_d.tile([128, 512], F32, tag=f"d{i}", name=f"dps{i}")
                       for i in range(4)]

                for c in range(NCH):
                    htc = htp.tile([128, 4 * TB], BF16)
                    for m in range(4):
                        g1 = ps_g1.tile([128, TB], F32, tag="g1")
                        col0 = c * CHW + m * 128
                        for k in range(KD):
                            nc.tensor.matmul(
                                g1[:],
                                lhsT=wdn_sb[:, k * EH + col0:k * EH + col0 + 128],
                                rhs=xtb[:, k * TB:(k + 1) * TB],
                                start=(k == 0), stop=(k == KD - 1))
                        eh_t = c * 4 + m
                        hslice = htc[:, m * TB:(m + 1) * TB]
                        nc.scalar.activation(hslice, g1[:], AF.Gelu,
                                             bias=dbt_sb[:, eh_t:eh_t + 1])
                        e_idx = (c * CHW + m * 128) // HID
                        nc.vector.tensor_scalar_mul(
                            hslice, hslice, wsb[b][:, e_idx:e_idx + 1])
                    for tt in range(2):
                        for m in range(4):
                            lhs = htc[:, m * TB + tt * 128:m * TB + (tt + 1) * 128]
                            kk = c * 4 + m
                            for hh in range(2):
                                nc.tensor.matmul(
                                    dps[tt * 2 + hh][:], lhsT=lhs,
                                    rhs=wup_sb[:, kk * DM + hh * 512:
                                               kk * DM + (hh + 1) * 512],
                                    start=(c == 0 and m == 0),
                                    stop=False)

                # up_b bias term closes each accumulation group (keeps block 0
                # off the router's critical path)
                for tt in range(2):
                    for hh in range(2):
                        nc.tensor.matmul(
                            dps[tt * 2 + hh][:], lhsT=ones_bf[:],
                            rhs=bias_bf[b][0:1, hh * 512:(hh + 1) * 512],
                            start=False, stop=True)

                outb = outp.tile([128, 2 * DM], F32)
                for tt in range(2):
                    for hh in range(2):
                        sl = slice(tt * DM + hh * 512, tt * DM + (hh + 1) * 512)
                        nc.vector.tensor_tensor(
                            out=outb[:, sl], in0=dps[tt * 2 + hh][:],
                            in1=xrb[:, sl], op=ALU.add)
                for tt in range(2):
                    nc.sync.dma_start(
                        out=out[t0 + tt * 128:t0 + (tt + 1) * 128, :],
                        in_=outb[:, tt * DM:(tt + 1) * DM])

    nc.compile()
    return nc


def _get_nc():
    if "nc" not in _CACHE:
        _CACHE["nc"] = _build()
    return _CACHE["nc"]


def _softmax32(x):
    x = np.asarray(x, np.float32)
    m = x.max(axis=-1, keepdims=True)
    e = np.exp(x - m)
    return e / e.sum(axis=-1, keepdims=True)


def kernel(**inputs):
    global LAST_RESULT
    features = np.ascontiguousarray(np.asarray(inputs["features"], np.float32))
    x_raw = np.asarray(inputs["x_raw"], np.float32)
    down_w = np.asarray(inputs["down_w"], np.float32)
    down_b = np.asarray(inputs["down_b"], np.float32)
    up_w = np.asarray(inputs["up_w"], np.float32)
    up_b = np.asarray(inputs["up_b"], np.float32)
    cond_w = np.asarray(inputs["cond_w"], np.float32)
    cond_b = np.asarray(inputs["cond_b"], np.float32)
    stage_w = np.asarray(inputs["stage_w"], np.float32)
    stage_b = np.asarray(inputs["stage_b"], np.float32)

    # host-side router for the tiny stage branch (16x2048x14 input)
    stage_logits = x_raw.mean(axis=1, dtype=np.float32) @ stage_w + stage_b
    stage_weights = _softmax32(stage_logits)            # [B, G]

    wdn_np = np.ascontiguousarray(
        down_w.transpose(1, 0, 2).reshape(DM, EH)).astype(BF16_NP)
    wup_np = np.ascontiguousarray(up_w.reshape(EH, DM)).astype(BF16_NP)
    dbt_np = np.ascontiguousarray(down_b.reshape(EH).reshape(NKEH, 128).T)
    upb_np = np.ascontiguousarray(up_b)                  # [E, DM]
    cws_np = np.ascontiguousarray(cond_w / np.float32(S))
    cb_np = cond_b.reshape(1, C)

    nc = _get_nc()
    in_maps = []
    for cidx in range(NCORE):
        fs = features[BPC * cidx:BPC * (cidx + 1)].reshape(TPC, DM)
        in_maps.append({
            "xt": fs.T.astype(BF16_NP),
            "xres": fs,
            "wdn": wdn_np,
            "wup": wup_np,
            "dbt": dbt_np,
            "upb": upb_np,
            "cws": cws_np,
            "cb": cb_np,
            "sw": np.ascontiguousarray(
                stage_weights[BPC * cidx:BPC * (cidx + 1)].reshape(1, BPC * G)),
        })

    res = run_bass_kernel_spmd(nc, in_maps, list(range(NCORE)))
    LAST_RESULT = res

    output = np.concatenate(
        [res.results[c]["out"] for c in range(NCORE)], axis=0
    ).reshape(B, S, DM)
    cond_logits = np.concatenate(
        [res.results[c]["clg"] for c in range(NCORE)], axis=0)   # [B, C]
    cond_weights = _softmax32(cond_logits)

    joint = cond_weights[:, :, None] * stage_weights[:, None, :]
    flat = joint.reshape(B, E).astype(np.float32)
    expert_loads = flat.mean(axis=0, dtype=np.float32)
    lb_loss = np.float32(E * np.sum(expert_loads * expert_loads,
                                    dtype=np.float32) * np.float32(0.01))

    return (output.astype(np.float32), cond_weights.astype(np.float32),
            stage_weights.astype(np.float32), expert_loads, lb_loss)


# revision 8
# speedup vs baseline: 1.0707x; 1.0079x over previous
"""Trainium2 Bass kernel for the DSA-MoE routing module.

Strategy: data-parallel over batch. Each of the 8 NeuronCores gets 2 full
batches (4096 tokens). Expert weights are replicated, cast to bf16 on host,
and kept SBUF-resident. Per core:

  - cond router: sum features over seq via DVE reduce on the pre-transposed
    bf16 activations, then a small fp32 matmul against cond_w/2048 (+cond_b
    via a rank-1 matmul). Final softmax happens on host from the returned
    logits; an on-device softmax produces the internal routing weights.
  - stage router: x_raw is tiny (16x2048x14) -> host computes stage softmax
    and passes the per-batch weights in.
  - main pipeline per 256-token block: GEMM1 (X.T @ W_down -> H.T in
    [EH, tokens] layout, bf16, PSUM fp32), fused Gelu+down_b eviction on
    ACT, per-(batch,expert) scale on DVE, GEMM2 accumulates
    delta = Hw.T^T @ W_up directly in PSUM across all 9 EH-chunks
    (plus a rank-1 matmul adding the up_b bias term), final eviction adds
    the fp32 residual.

The full (unsharded) inputs come in; sharding/gather happens on host.
"""

import sys

sys.path.insert(0, "/opt/trn_rl_repo")

from contextlib import ExitStack

import ml_dtypes
import numpy as np

import concourse.bass as bass  # noqa: F401  (registers bass types)
import concourse.tile as tile
from concourse import bacc, mybir
from concourse.bass_utils import run_bass_kernel_spmd

BF16, F32 = mybir.dt.bfloat16, mybir.dt.float32
AF = mybir.ActivationFunctionType
AX = mybir.AxisListType
ALU = mybir.AluOpType
BF16_NP = ml_dtypes.bfloat16

B, S, DM, HID = 16, 2048, 1024, 256
C, G = 6, 3
E = C * G                      # 18 experts
EH = E * HID                   # 4608
NCORE = 8
BPC = B // NCORE               # batches per core = 2
TPC = BPC * S                  # tokens per core = 4096
TB = 256                       # token block
NBLK = S // TB                 # blocks per batch = 8
KD = DM // 128                 # 8 k-tiles over D
NCH = 9                        # EH chunks
CHW = EH // NCH                # 512 EH cols per chunk
NKEH = EH // 128               # 36 EH k-tiles

_CACHE = {}
LAST_RESULT = None


def _build():
    nc = bacc.Bacc("TRN2", target_bir_lowering=False, debug=False,
                   num_devices=NCORE)
    xt = nc.dram_tensor("xt", [DM, TPC], BF16, kind="ExternalInput").ap()
    xres = nc.dram_tensor("xres", [TPC, DM], F32, kind="ExternalInput").ap()
    wdn = nc.dram_tensor("wdn", [DM, EH], BF16, kind="ExternalInput").ap()
    wup = nc.dram_tensor("wup", [EH, DM], BF16, kind="ExternalInput").ap()
    dbt = nc.dram_tensor("dbt", [128, NKEH], F32, kind="ExternalInput").ap()
    upb = nc.dram_tensor("upb", [E, DM], F32, kind="ExternalInput").ap()
    cws = nc.dram_tensor("cws", [DM, C], F32, kind="ExternalInput").ap()
    cb = nc.dram_tensor("cb", [1, C], F32, kind="ExternalInput").ap()
    sw = nc.dram_tensor("sw", [1, BPC * G], F32, kind="ExternalInput").ap()
    out = nc.dram_tensor("out", [TPC, DM], F32, kind="ExternalOutput").ap()
    clg = nc.dram_tensor("clg", [BPC, C], F32, kind="ExternalOutput").ap()

    with tile.TileContext(nc) as tc, ExitStack() as ctx:
        const = ctx.enter_context(tc.tile_pool(name="const", bufs=1))
        wpool = ctx.enter_context(tc.tile_pool(name="wpool", bufs=1))
        xtp = ctx.enter_context(tc.tile_pool(name="xtp", bufs=2))
        xrp = ctx.enter_context(tc.tile_pool(name="xrp", bufs=1))
        htp = ctx.enter_context(tc.tile_pool(name="htp", bufs=4))
        outp = ctx.enter_context(tc.tile_pool(name="outp", bufs=1))
        ps_g1 = ctx.enter_context(tc.tile_pool(name="psg1", bufs=3, space="PSUM"))
        ps_d = ctx.enter_context(tc.tile_pool(name="psd", bufs=1, space="PSUM"))
        ps_s = ctx.enter_context(tc.tile_pool(name="pss", bufs=1, space="PSUM"))

        # DMA issue order is roughly execution order: wdn + block-0
        # activations first (gives PE its runway), then the router means,
        # then wup (trickles in under block 0's GEMM1), then the rest.
        wdn_sb = wpool.tile([128, KD * EH], BF16)       # k-tile k at [:, k*EH:]
        for k in range(KD):
            nc.sync.dma_start(out=wdn_sb[:, k * EH:(k + 1) * EH],
                              in_=wdn[k * 128:(k + 1) * 128, :])
        xtb0 = xtp.tile([128, KD * TB], BF16, tag="xtm", name="xtb0")
        for k in range(KD):
            nc.sync.dma_start(out=xtb0[:, k * TB:(k + 1) * TB],
                              in_=xt[k * 128:(k + 1) * 128, 0:TB])
        dbt_sb = const.tile([128, NKEH], F32)
        nc.sync.dma_start(out=dbt_sb[:], in_=dbt[:])
        cws_sb = const.tile([128, KD * C], F32)
        for k in range(KD):
            nc.sync.dma_start(out=cws_sb[:, k * C:(k + 1) * C],
                              in_=cws[k * 128:(k + 1) * 128, :])
        cb_sb = const.tile([1, C], F32)
        nc.sync.dma_start(out=cb_sb[:], in_=cb[:])
        sw_sb = const.tile([1, BPC * G], F32)
        nc.sync.dma_start(out=sw_sb[:], in_=sw[:])
        ones_f = const.tile([1, 128], F32)
        nc.any.memset(ones_f[:], 1.0)
        ones_bf = const.tile([1, 128], BF16)
        nc.any.memset(ones_bf[:], 1.0)

        # chunk-0 up-weights early so block-0 GEMM2 isn't starved
        wup_sb = wpool.tile([128, NKEH * DM], BF16)     # k-tile kk at [:, kk*DM:]
        for kk in range(4):
            nc.sync.dma_start(out=wup_sb[:, kk * DM:(kk + 1) * DM],
                              in_=wup[kk * 128:(kk + 1) * 128, :])
        upb_sb = const.tile([E, DM], F32)
        nc.sync.dma_start(out=upb_sb[:], in_=upb[:])

        # ---- feature means + routers, batch-0 first ---------------------
        mf_sb = const.tile([128, BPC * KD], F32)

        def batch_means(b):
            for k in range(KD):
                xtm = xtp.tile([128, S], BF16, tag="xmean", name=f"xtm{b}{k}")
                nc.sync.dma_start(out=xtm[:],
                                  in_=xt[k * 128:(k + 1) * 128,
                                         b * S:(b + 1) * S])
                nc.vector.reduce_sum(mf_sb[:, b * KD + k:b * KD + k + 1],
                                     xtm[:], axis=AX.X)

        wsb = [None] * BPC
        bias_bf = [None] * BPC

        def batch_router(b):
            lgp = ps_s.tile([1, C], F32, tag="small")
            for k in range(KD):
                nc.tensor.matmul(lgp[:], lhsT=mf_sb[:, b * KD + k:b * KD + k + 1],
                                 rhs=cws_sb[:, k * C:(k + 1) * C],
                                 start=(k == 0), stop=False)
            nc.tensor.matmul(lgp[:], lhsT=ones_f[0:1, 0:1], rhs=cb_sb[:],
                             start=False, stop=True)
            lg_sb = const.tile([1, C], F32, tag=f"lg{b}")
            nc.scalar.activation(lg_sb[:], lgp[:], AF.Copy)
            nc.sync.dma_start(out=clg[b:b + 1, :], in_=lg_sb[:])

            e_sb = const.tile([1, C], F32, tag=f"esb{b}")
            nc.scalar.activation(e_sb[:], lg_sb[:], AF.Exp)
            s_sb = const.tile([1, 1], F32, tag=f"ssb{b}")
            nc.vector.reduce_sum(s_sb[:], e_sb[:], axis=AX.X)
            r_sb = const.tile([1, 1], F32, tag=f"rsb{b}")
            nc.vector.reciprocal(r_sb[:], s_sb[:])
            cwd = const.tile([1, C], F32, tag=f"cwd{b}")
            nc.vector.tensor_scalar_mul(cwd[:], e_sb[:], r_sb[0:1, 0:1])

            # flat joint weights fw[e = c*G + g] = cond[c] * stage[g]
            fw = const.tile([1, E], F32, tag=f"fw{b}")
            fw3 = fw[0:1, :].rearrange("p (c g) -> p c g", g=G)
            cwd3 = cwd[0:1, :].rearrange("p (c u) -> p c u", u=1)
            for g in range(G):
                nc.vector.tensor_scalar_mul(fw3[:, :, g:g + 1], cwd3[:],
                                            sw_sb[0:1, b * G + g:b * G + g + 1])

            # broadcast across partitions: [128, E]
            wbp = ps_s.tile([128, E], F32, tag="small")
            nc.tensor.matmul(wbp[:], lhsT=ones_f[:], rhs=fw[:],
                             start=True, stop=True)
            ws = const.tile([128, E], F32, tag=f"ws{b}")
            nc.scalar.activation(ws[:], wbp[:], AF.Copy)
            wsb[b] = ws

            # fw -> column layout [E, 1] via DVE 32x32 transpose
            tin = const.tile([32, 32], F32, tag=f"ti{b}")
            nc.any.memset(tin[:], 0.0)
            nc.vector.tensor_copy(tin[0:1, 0:E], fw[:])
            tco = const.tile([32, 32], F32, tag=f"tc{b}")
            nc.vector.transpose(tco[:], tin[:])

            # bias_term = fw @ up_b   -> [1, DM], cast to bf16
            bb = const.tile([1, DM], BF16, tag=f"bb{b}")
            for hh in range(2):
                bps = ps_s.tile([1, 512], F32, tag="small")
                nc.tensor.matmul(bps[:], lhsT=tco[0:E, 0:1],
                                 rhs=upb_sb[0:E, hh * 512:(hh + 1) * 512],
                                 start=True, stop=True)
                nc.scalar.activation(bb[0:1, hh * 512:(hh + 1) * 512], bps[:],
                                     AF.Copy)
            bias_bf[b] = bb

        batch_means(0)
        batch_router(0)
        # remaining up-weights trickle in behind block 0's first chunks
        for kk in range(4, NKEH):
            nc.sync.dma_start(out=wup_sb[:, kk * DM:(kk + 1) * DM],
                              in_=wup[kk * 128:(kk + 1) * 128, :])
        batch_means(1)
        batch_router(1)
        xrb0 = xrp.tile([128, 2 * DM], F32, name="xrb0")
        for tt in range(2):
            nc.sync.dma_start(out=xrb0[:, tt * DM:(tt + 1) * DM],
                              in_=xres[tt * 128:(tt + 1) * 128, :])

        # ---- main pipeline ----------------------------------------------
        for b in range(BPC):
            for blk in range(NBLK):
                t0 = b * S + blk * TB
                if b == 0 and blk == 0:
                    xtb, xrb = xtb0, xrb0
                else:
                    xtb = xtp.tile([128, KD * TB], BF16, tag="xtm")
                    for k in range(KD):
                        nc.sync.dma_start(
                            out=xtb[:, k * TB:(k + 1) * TB],
                            in_=xt[k * 128:(k + 1) * 128, t0:t0 + TB])
                    xrb = xrp.tile([128, 2 * DM], F32)
                    for tt in range(2):
                        nc.sync.dma_start(
                            out=xrb[:, tt * DM:(tt + 1) * DM],
                            in_=xres[t0 + tt * 128:t0 + (tt + 1) * 128, :])

                dps = [ps_d.tile([128, 512], F32, tag=f"d{i}", name=f"dps{i}")
                       for i in range(4)]

                for c in range(NCH):
                    htc = htp.tile([128, 4 * TB], BF16)
                    for m in range(4):
                        g1 = ps_g1.tile([128, TB], F32, tag="g1")
                        col0 = c * CHW + m * 128
                        for k in range(KD):
                            nc.tensor.matmul(
                                g1[:],
                                lhsT=wdn_sb[:, k * EH + col0:k * EH + col0 + 128],
                                rhs=xtb[:, k * TB:(k + 1) * TB],
                                start=(k == 0), stop=(k == KD - 1))
                        eh_t = c * 4 + m
                        hslice = htc[:, m * TB:(m + 1) * TB]
                        nc.scalar.activation(hslice, g1[:], AF.Gelu,
                                             bias=dbt_sb[:, eh_t:eh_t + 1])
                        e_idx = (c * CHW + m * 128) // HID
                        nc.vector.tensor_scalar_mul(
                            hslice, hslice, wsb[b][:, e_idx:e_idx + 1])
                    for tt in range(2):
                        for m in range(4):
                            lhs = htc[:, m * TB + tt * 128:m * TB + (tt + 1) * 128]
                            kk = c * 4 + m
                            for hh in range(2):
                                nc.tensor.matmul(
                                    dps[tt * 2 + hh][:], lhsT=lhs,
                                    rhs=wup_sb[:, kk * DM + hh * 512:
                                               kk * DM + (hh + 1) * 512],
                                    start=(c == 0 and m == 0),
                                    stop=False)

                # up_b bias term closes each accumulation group (keeps block 0
                # off the router's critical path)
                for tt in range(2):
                    for hh in range(2):
                        nc.tensor.matmul(
                            dps[tt * 2 + hh][:], lhsT=ones_bf[:],
                            rhs=bias_bf[b][0:1, hh * 512:(hh + 1) * 512],
                            start=False, stop=True)

                outb = outp.tile([128, 2 * DM], F32)
                for tt in range(2):
                    for hh in range(2):
                        sl = slice(tt * DM + hh * 512, tt * DM + (hh + 1) * 512)
                        nc.vector.tensor_tensor(
                            out=outb[:, sl], in0=dps[tt * 2 + hh][:],
                            in1=xrb[:, sl], op=ALU.add)
                for tt in range(2):
                    nc.sync.dma_start(
                        out=out[t0 + tt * 128:t0 + (tt + 1) * 128, :],
                        in_=outb[:, tt * DM:(tt + 1) * DM])

    nc.compile()
    return nc


def _get_nc():
    if "nc" not in _CACHE:
        _CACHE["nc"] = _build()
    return _CACHE["nc"]


def _softmax32(x):
    x = np.asarray(x, np.float32)
    m = x.max(axis=-1, keepdims=True)
    e = np.exp(x - m)
    return e / e.sum(axis=-1, keepdims=True)


def kernel(**inputs):
    global LAST_RESULT
    features = np.ascontiguousarray(np.asarray(inputs["features"], np.float32))
    x_raw = np.asarray(inputs["x_raw"], np.float32)
    down_w = np.asarray(inputs["down_w"], np.float32)
    down_b = np.asarray(inputs["down_b"], np.float32)
    up_w = np.asarray(inputs["up_w"], np.float32)
    up_b = np.asarray(inputs["up_b"], np.float32)
    cond_w = np.asarray(inputs["cond_w"], np.float32)
    cond_b = np.asarray(inputs["cond_b"], np.float32)
    stage_w = np.asarray(inputs["stage_w"], np.float32)
    stage_b = np.asarray(inputs["stage_b"], np.float32)

    # host-side router for the tiny stage branch (16x2048x14 input)
    stage_logits = x_raw.mean(axis=1, dtype=np.float32) @ stage_w + stage_b
    stage_weights = _softmax32(stage_logits)            # [B, G]

    wdn_np = np.ascontiguousarray(
        down_w.transpose(1, 0, 2).reshape(DM, EH)).astype(BF16_NP)
    wup_np = np.ascontiguousarray(up_w.reshape(EH, DM)).astype(BF16_NP)
    dbt_np = np.ascontiguousarray(down_b.reshape(EH).reshape(NKEH, 128).T)
    upb_np = np.ascontiguousarray(up_b)                  # [E, DM]
    cws_np = np.ascontiguousarray(cond_w / np.float32(S))
    cb_np = cond_b.reshape(1, C)

    nc = _get_nc()
    in_maps = []
    for cidx in range(NCORE):
        fs = features[BPC * cidx:BPC * (cidx + 1)].reshape(TPC, DM)
        in_maps.append({
            "xt": fs.T.astype(BF16_NP),
            "xres": fs,
            "wdn": wdn_np,
            "wup": wup_np,
            "dbt": dbt_np,
            "upb": upb_np,
            "cws": cws_np,
            "cb": cb_np,
            "sw": np.ascontiguousarray(
                stage_weights[BPC * cidx:BPC * (cidx + 1)].reshape(1, BPC * G)),
        })

    res = run_bass_kernel_spmd(nc, in_maps, list(range(NCORE)))
    LAST_RESULT = res

    output = np.concatenate(
        [res.results[c]["out"] for c in range(NCORE)], axis=0
    ).reshape(B, S, DM)
    cond_logits = np.concatenate(
        [res.results[c]["clg"] for c in range(NCORE)], axis=0)   # [B, C]
    cond_weights = _softmax32(cond_logits)

    joint = cond_weights[:, :, None] * stage_weights[:, None, :]
    flat = joint.reshape(B, E).astype(np.float32)
    expert_loads = flat.mean(axis=0, dtype=np.float32)
    lb_loss = np.float32(E * np.sum(expert_loads * expert_loads,
                                    dtype=np.float32) * np.float32(0.01))

    return (output.astype(np.float32), cond_weights.astype(np.float32),
            stage_weights.astype(np.float32), expert_loads, lb_loss)


# revision 17
# speedup vs baseline: 1.3315x; 1.2435x over previous
"""Trainium2 Bass kernel for the DSA-MoE routing module.

Strategy: data-parallel over batch. Each of the 8 NeuronCores gets 2 full
batches (4096 tokens). Expert weights are replicated, cast to bf16 on host,
and kept SBUF-resident. Per core:

  - cond router: sum features over seq via DVE reduce on the pre-transposed
    bf16 activations, then a small fp32 matmul against cond_w/2048 (+cond_b
    via a rank-1 matmul). Final softmax happens on host from the returned
    logits; an on-device softmax produces the internal routing weights.
  - stage router: x_raw is tiny (16x2048x14) -> host computes stage softmax
    and passes the per-batch weights in.
  - main pipeline per 256-token block: GEMM1 (X.T @ W_down -> H.T in
    [EH, tokens] layout, bf16, PSUM fp32), fused Gelu+down_b eviction on
    ACT, per-(batch,expert) scale on DVE, GEMM2 accumulates
    delta = Hw.T^T @ W_up directly in PSUM across all 9 EH-chunks
    (plus a rank-1 matmul adding the up_b bias term), final eviction adds
    the fp32 residual.

The full (unsharded) inputs come in; sharding/gather happens on host.
"""

import sys

sys.path.insert(0, "/opt/trn_rl_repo")

from contextlib import ExitStack

import ml_dtypes
import numpy as np

import concourse.bass as bass  # noqa: F401  (registers bass types)
import concourse.tile as tile
from concourse import bacc, mybir
from concourse.bass_utils import run_bass_kernel_spmd

BF16, F32 = mybir.dt.bfloat16, mybir.dt.float32
FP8 = mybir.dt.float8e4
AF = mybir.ActivationFunctionType
AX = mybir.AxisListType
ALU = mybir.AluOpType
BF16_NP = ml_dtypes.bfloat16
FP8_NP = ml_dtypes.float8_e4m3

# fp8 GEMM2: h is pre-scaled by HS (off the e4m3 subnormal range) and W_up by
# WS; the final eviction divides the PSUM result by HS*WS, with the residual
# and bias terms pre-multiplied to match.
G2_FP8 = True
HS = 128.0
WS = 64.0
TOT = HS * WS

B, S, DM, HID = 16, 2048, 1024, 256
C, G = 6, 3
E = C * G                      # 18 experts
EH = E * HID                   # 4608
NCORE = 8
BPC = B // NCORE               # batches per core = 2
TPC = BPC * S                  # tokens per core = 4096
TB = 256                       # token block
NBLK = S // TB                 # blocks per batch = 8
KD = DM // 128                 # 8 k-tiles over D
NCH = 9                        # EH chunks
CHW = EH // NCH                # 512 EH cols per chunk
NKEH = EH // 128               # 36 EH k-tiles

_CACHE = {}
LAST_RESULT = None


def _build():
    nc = bacc.Bacc("TRN2", target_bir_lowering=False, debug=False,
                   num_devices=NCORE)
    xt = nc.dram_tensor("xt", [DM, TPC], BF16, kind="ExternalInput").ap()
    xres = nc.dram_tensor("xres", [TPC, DM], F32, kind="ExternalInput").ap()
    wdn = nc.dram_tensor("wdn", [DM, EH], BF16, kind="ExternalInput").ap()
    wup = nc.dram_tensor("wup", [EH, DM], FP8 if G2_FP8 else BF16,
                         kind="ExternalInput").ap()
    dbt = nc.dram_tensor("dbt", [128, NKEH], F32, kind="ExternalInput").ap()
    upb = nc.dram_tensor("upb", [E, DM], F32, kind="ExternalInput").ap()
    cws = nc.dram_tensor("cws", [DM, C], F32, kind="ExternalInput").ap()
    cb = nc.dram_tensor("cb", [1, C], F32, kind="ExternalInput").ap()
    sw = nc.dram_tensor("sw", [1, BPC * G], F32, kind="ExternalInput").ap()
    out = nc.dram_tensor("out", [TPC, DM], F32, kind="ExternalOutput").ap()
    clg = nc.dram_tensor("clg", [BPC, C], F32, kind="ExternalOutput").ap()

    with tile.TileContext(nc) as tc, ExitStack() as ctx:
        const = ctx.enter_context(tc.tile_pool(name="const", bufs=1))
        wpool = ctx.enter_context(tc.tile_pool(name="wpool", bufs=1))
        xtp = ctx.enter_context(tc.tile_pool(name="xtp", bufs=2))
        xrp = ctx.enter_context(tc.tile_pool(name="xrp", bufs=1))
        htp = ctx.enter_context(tc.tile_pool(name="htp", bufs=4))
        outp = ctx.enter_context(tc.tile_pool(name="outp", bufs=1))
        ps_g1 = ctx.enter_context(tc.tile_pool(name="psg1", bufs=3, space="PSUM"))
        ps_d = ctx.enter_context(tc.tile_pool(name="psd", bufs=1, space="PSUM"))
        ps_s = ctx.enter_context(tc.tile_pool(name="pss", bufs=1, space="PSUM"))

        # DMA issue order is roughly execution order: wdn + block-0
        # activations first (gives PE its runway), then the router means,
        # then wup (trickles in under block 0's GEMM1), then the rest.
        wdn_sb = wpool.tile([128, KD * EH], BF16)       # k-tile k at [:, k*EH:]
        for k in range(KD):
            nc.sync.dma_start(out=wdn_sb[:, k * EH:(k + 1) * EH],
                              in_=wdn[k * 128:(k + 1) * 128, :])
        xtb0 = xtp.tile([128, KD * TB], BF16, tag="xtm", name="xtb0")
        for k in range(KD):
            nc.sync.dma_start(out=xtb0[:, k * TB:(k + 1) * TB],
                              in_=xt[k * 128:(k + 1) * 128, 0:TB])
        dbt_sb = const.tile([128, NKEH], F32)
        nc.sync.dma_start(out=dbt_sb[:], in_=dbt[:])
        cws_sb = const.tile([128, KD * C], F32)
        for k in range(KD):
            nc.sync.dma_start(out=cws_sb[:, k * C:(k + 1) * C],
                              in_=cws[k * 128:(k + 1) * 128, :])
        cb_sb = const.tile([1, C], F32)
        nc.sync.dma_start(out=cb_sb[:], in_=cb[:])
        sw_sb = const.tile([1, BPC * G], F32)
        nc.sync.dma_start(out=sw_sb[:], in_=sw[:])
        ones_f = const.tile([1, 128], F32)
        nc.any.memset(ones_f[:], 1.0)
        ones_bf = const.tile([1, 128], BF16)
        nc.any.memset(ones_bf[:], 1.0)

        # chunk-0 up-weights early so block-0 GEMM2 isn't starved
        wup_sb = wpool.tile([128, NKEH * DM], FP8 if G2_FP8 else BF16)
        for kk in range(4):
            nc.sync.dma_start(out=wup_sb[:, kk * DM:(kk + 1) * DM],
                              in_=wup[kk * 128:(kk + 1) * 128, :])
        upb_sb = const.tile([E, DM], F32)
        nc.sync.dma_start(out=upb_sb[:], in_=upb[:])

        # ---- feature means + routers, batch-0 first ---------------------
        mf_sb = const.tile([128, BPC * KD], F32)

        def batch_means(b):
            for k in range(KD):
                xtm = xtp.tile([128, S], BF16, tag="xmean", name=f"xtm{b}{k}")
                nc.sync.dma_start(out=xtm[:],
                                  in_=xt[k * 128:(k + 1) * 128,
                                         b * S:(b + 1) * S])
                nc.vector.reduce_sum(mf_sb[:, b * KD + k:b * KD + k + 1],
                                     xtm[:], axis=AX.X)

        wsb = [None] * BPC
        bias_bf = [None] * BPC

        def batch_router(b):
            lgp = ps_s.tile([1, C], F32, tag="small")
            for k in range(KD):
                nc.tensor.matmul(lgp[:], lhsT=mf_sb[:, b * KD + k:b * KD + k + 1],
                                 rhs=cws_sb[:, k * C:(k + 1) * C],
                                 start=(k == 0), stop=False)
            nc.tensor.matmul(lgp[:], lhsT=ones_f[0:1, 0:1], rhs=cb_sb[:],
                             start=False, stop=True)
            lg_sb = const.tile([1, C], F32, tag=f"lg{b}")
            nc.scalar.activation(lg_sb[:], lgp[:], AF.Copy)
            nc.sync.dma_start(out=clg[b:b + 1, :], in_=lg_sb[:])

            e_sb = const.tile([1, C], F32, tag=f"esb{b}")
            nc.scalar.activation(e_sb[:], lg_sb[:], AF.Exp)
            s_sb = const.tile([1, 1], F32, tag=f"ssb{b}")
            nc.vector.reduce_sum(s_sb[:], e_sb[:], axis=AX.X)
            r_sb = const.tile([1, 1], F32, tag=f"rsb{b}")
            nc.vector.reciprocal(r_sb[:], s_sb[:])
            cwd = const.tile([1, C], F32, tag=f"cwd{b}")
            nc.vector.tensor_scalar_mul(cwd[:], e_sb[:], r_sb[0:1, 0:1])

            # flat joint weights fw[e = c*G + g] = cond[c] * stage[g]
            fw = const.tile([1, E], F32, tag=f"fw{b}")
            fw3 = fw[0:1, :].rearrange("p (c g) -> p c g", g=G)
            cwd3 = cwd[0:1, :].rearrange("p (c u) -> p c u", u=1)
            for g in range(G):
                nc.vector.tensor_scalar_mul(fw3[:, :, g:g + 1], cwd3[:],
                                            sw_sb[0:1, b * G + g:b * G + g + 1])

            # broadcast across partitions: [128, E]
            wbp = ps_s.tile([128, E], F32, tag="small")
            nc.tensor.matmul(wbp[:], lhsT=ones_f[:], rhs=fw[:],
                             start=True, stop=True)
            ws = const.tile([128, E], F32, tag=f"ws{b}")
            nc.scalar.activation(ws[:], wbp[:], AF.Copy,
                                 scale=HS if G2_FP8 else 1.0)
            wsb[b] = ws

            # fw -> column layout [E, 1] via DVE 32x32 transpose
            tin = const.tile([32, 32], F32, tag=f"ti{b}")
            nc.any.memset(tin[:], 0.0)
            nc.vector.tensor_copy(tin[0:1, 0:E], fw[:])
            tco = const.tile([32, 32], F32, tag=f"tc{b}")
            nc.vector.transpose(tco[:], tin[:])

            # bias_term = fw @ up_b   -> [1, DM], cast to bf16
            bb = const.tile([1, DM], BF16, tag=f"bb{b}")
            for hh in range(2):
                bps = ps_s.tile([1, 512], F32, tag="small")
                nc.tensor.matmul(bps[:], lhsT=tco[0:E, 0:1],
                                 rhs=upb_sb[0:E, hh * 512:(hh + 1) * 512],
                                 start=True, stop=True)
                nc.scalar.activation(bb[0:1, hh * 512:(hh + 1) * 512], bps[:],
                                     AF.Copy, scale=TOT if G2_FP8 else 1.0)
            bias_bf[b] = bb

        batch_means(0)
        batch_router(0)
        # remaining up-weights trickle in behind block 0's first chunks
        for kk in range(4, NKEH):
            nc.sync.dma_start(out=wup_sb[:, kk * DM:(kk + 1) * DM],
                              in_=wup[kk * 128:(kk + 1) * 128, :])
        batch_means(1)
        batch_router(1)
        xrb0 = xrp.tile([128, 2 * DM], F32, name="xrb0")
        for tt in range(2):
            nc.sync.dma_start(out=xrb0[:, tt * DM:(tt + 1) * DM],
                              in_=xres[tt * 128:(tt + 1) * 128, :])

        # ---- main pipeline ----------------------------------------------
        for b in range(BPC):
            for blk in range(NBLK):
                t0 = b * S + blk * TB
                if b == 0 and blk == 0:
                    xtb, xrb = xtb0, xrb0
                else:
                    xtb = xtp.tile([128, KD * TB], BF16, tag="xtm")
                    for k in range(KD):
                        nc.sync.dma_start(
                            out=xtb[:, k * TB:(k + 1) * TB],
                            in_=xt[k * 128:(k + 1) * 128, t0:t0 + TB])
                    xrb = xrp.tile([128, 2 * DM], F32)
                    for tt in range(2):
                        nc.sync.dma_start(
                            out=xrb[:, tt * DM:(tt + 1) * DM],
                            in_=xres[t0 + tt * 128:t0 + (tt + 1) * 128, :])

                dps = [ps_d.tile([128, 512], F32, tag=f"d{i}", name=f"dps{i}")
                       for i in range(4)]

                for c in range(NCH):
                    htc = htp.tile([128, 4 * TB], FP8 if G2_FP8 else BF16)
                    for m in range(4):
                        g1 = ps_g1.tile([128, TB], F32, tag="g1")
                        col0 = c * CHW + m * 128
                        for k in range(KD):
                            nc.tensor.matmul(
                                g1[:],
                                lhsT=wdn_sb[:, k * EH + col0:k * EH + col0 + 128],
                                rhs=xtb[:, k * TB:(k + 1) * TB],
                                start=(k == 0), stop=(k == KD - 1))
                        eh_t = c * 4 + m
                        hslice = htc[:, m * TB:(m + 1) * TB]
                        if G2_FP8:
                            htmp = htp.tile([128, TB], BF16, tag="htmp")
                            nc.scalar.activation(htmp[:], g1[:], AF.Gelu,
                                                 bias=dbt_sb[:, eh_t:eh_t + 1])
                            hsrc = htmp[:]
                        else:
                            nc.scalar.activation(hslice, g1[:], AF.Gelu,
                                                 bias=dbt_sb[:, eh_t:eh_t + 1])
                            hsrc = hslice
                        e_idx = (c * CHW + m * 128) // HID
                        nc.vector.tensor_scalar_mul(
                            hslice, hsrc, wsb[b][:, e_idx:e_idx + 1])
                    if G2_FP8:
                        ht3 = htc[:, :].rearrange("p (m t) -> p m t", m=4)
                        wu3 = wup_sb[:, :].rearrange("p (kk dd) -> p kk dd",
                                                     kk=NKEH)
                        for tt in range(2):
                            for m in (0, 2):
                                kk = c * 4 + m
                                for hh in range(2):
                                    nc.tensor.matmul(
                                        dps[tt * 2 + hh][:],
                                        lhsT=ht3[:, m:m + 2,
                                                 tt * 128:(tt + 1) * 128],
                                        rhs=wu3[:, kk:kk + 2,
                                                hh * 512:(hh + 1) * 512],
                                        start=(c == 0 and m == 0),
                                        stop=False,
                                        perf_mode=mybir.MatmulPerfMode.DoubleRow)
                    else:
                        for tt in range(2):
                            for m in range(4):
                                lhs = htc[:, m * TB + tt * 128:
                                          m * TB + (tt + 1) * 128]
                                kk = c * 4 + m
                                for hh in range(2):
                                    nc.tensor.matmul(
                                        dps[tt * 2 + hh][:], lhsT=lhs,
                                        rhs=wup_sb[:, kk * DM + hh * 512:
                                                   kk * DM + (hh + 1) * 512],
                                        start=(c == 0 and m == 0),
                                        stop=False)

                # up_b bias term closes each accumulation group (keeps block 0
                # off the router's critical path)
                for tt in range(2):
                    for hh in range(2):
                        nc.tensor.matmul(
                            dps[tt * 2 + hh][:], lhsT=ones_bf[:],
                            rhs=bias_bf[b][0:1, hh * 512:(hh + 1) * 512],
                            start=False, stop=True)

                outb = outp.tile([128, 2 * DM], F32)
                for tt in range(2):
                    for hh in range(2):
                        sl = slice(tt * DM + hh * 512, tt * DM + (hh + 1) * 512)
                        if G2_FP8:
                            # xres comes in pre-multiplied by TOT; fold the
                            # 1/TOT rescale into the ACT eviction
                            nc.vector.tensor_tensor(
                                out=dps[tt * 2 + hh][:], in0=dps[tt * 2 + hh][:],
                                in1=xrb[:, sl], op=ALU.add)
                            nc.scalar.activation(outb[:, sl],
                                                 dps[tt * 2 + hh][:],
                                                 AF.Copy, scale=1.0 / TOT)
                        else:
                            nc.vector.tensor_tensor(
                                out=outb[:, sl], in0=dps[tt * 2 + hh][:],
                                in1=xrb[:, sl], op=ALU.add)
                for tt in range(2):
                    nc.sync.dma_start(
                        out=out[t0 + tt * 128:t0 + (tt + 1) * 128, :],
                        in_=outb[:, tt * DM:(tt + 1) * DM])

    nc.compile()
    return nc


def _get_nc():
    if "nc" not in _CACHE:
        _CACHE["nc"] = _build()
    return _CACHE["nc"]


def _softmax32(x):
    x = np.asarray(x, np.float32)
    m = x.max(axis=-1, keepdims=True)
    e = np.exp(x - m)
    return e / e.sum(axis=-1, keepdims=True)


def kernel(**inputs):
    global LAST_RESULT
    features = np.ascontiguousarray(np.asarray(inputs["features"], np.float32))
    x_raw = np.asarray(inputs["x_raw"], np.float32)
    down_w = np.asarray(inputs["down_w"], np.float32)
    down_b = np.asarray(inputs["down_b"], np.float32)
    up_w = np.asarray(inputs["up_w"], np.float32)
    up_b = np.asarray(inputs["up_b"], np.float32)
    cond_w = np.asarray(inputs["cond_w"], np.float32)
    cond_b = np.asarray(inputs["cond_b"], np.float32)
    stage_w = np.asarray(inputs["stage_w"], np.float32)
    stage_b = np.asarray(inputs["stage_b"], np.float32)

    # host-side router for the tiny stage branch (16x2048x14 input)
    stage_logits = x_raw.mean(axis=1, dtype=np.float32) @ stage_w + stage_b
    stage_weights = _softmax32(stage_logits)            # [B, G]

    wdn_np = np.ascontiguousarray(
        down_w.transpose(1, 0, 2).reshape(DM, EH)).astype(BF16_NP)
    if G2_FP8:
        wup_np = (up_w.reshape(EH, DM) * np.float32(WS)).astype(FP8_NP)
    else:
        wup_np = np.ascontiguousarray(up_w.reshape(EH, DM)).astype(BF16_NP)
    dbt_np = np.ascontiguousarray(down_b.reshape(EH).reshape(NKEH, 128).T)
    upb_np = np.ascontiguousarray(up_b)                  # [E, DM]
    cws_np = np.ascontiguousarray(cond_w / np.float32(S))
    cb_np = cond_b.reshape(1, C)

    nc = _get_nc()
    in_maps = []
    for cidx in range(NCORE):
        fs = features[BPC * cidx:BPC * (cidx + 1)].reshape(TPC, DM)
        in_maps.append({
            "xt": fs.T.astype(BF16_NP),
            "xres": fs * np.float32(TOT) if G2_FP8 else fs,
            "wdn": wdn_np,
            "wup": wup_np,
            "dbt": dbt_np,
            "upb": upb_np,
            "cws": cws_np,
            "cb": cb_np,
            "sw": np.ascontiguousarray(
                stage_weights[BPC * cidx:BPC * (cidx + 1)].reshape(1, BPC * G)),
        })

    res = run_bass_kernel_spmd(nc, in_maps, list(range(NCORE)))
    LAST_RESULT = res

    output = np.concatenate(
        [res.results[c]["out"] for c in range(NCORE)], axis=0
    ).reshape(B, S, DM)
    cond_logits = np.concatenate(
        [res.results[c]["clg"] for c in range(NCORE)], axis=0)   # [B, C]
    cond_weights = _softmax32(cond_logits)

    joint = cond_weights[:, :, None] * stage_weights[:, None, :]
    flat = joint.reshape(B, E).astype(np.float32)
    expert_loads = flat.mean(axis=0, dtype=np.float32)
    lb_loss = np.float32(E * np.sum(expert_loads * expert_loads,
                                    dtype=np.float32) * np.float32(0.01))

    return (output.astype(np.float32), cond_weights.astype(np.float32),
            stage_weights.astype(np.float32), expert_loads, lb_loss)
